# revision 17
# baseline (speedup 1.0000x reference)
"""Grouped cross-attention Trainium2 kernel (v2, bf16).

Problem: B=4, SQ=1024, SK=2048, D=1024, H=16 heads (HD=64), G=4 groups
(GD=256) grouped o_proj, key/query masks, softmax over keys.

Sharding: 8 cores = (batch b = c//2) x (half of heads s = c%2).
Each core computes attention for 8 heads (= 2 o_proj groups) of one batch
and produces out[b, :, s*512:(s+1)*512].

v2 changes vs baseline:
  - all PE matmuls in bf16 (1 cycle/row vs ~2-4 for fp32r on HW)
  - score matmuls for a head PAIR run concurrently as row tiles
    (K=64 each, rows 0-63 / 64-127 of the PE array)
  - one exp ACT op per (pair, kchunk) covering both heads' score tiles
    ([128, 2, qn] strided over two PSUM banks), output bf16
  - no key-mask bias / no query-mask multiply: host compression zeroes
    padded K columns (exp(0)=1 harmless: padded V rows AND their ones-col
    are zero so they add nothing to numerator or denominator), and padded
    q columns are discarded by the host scatter
  - reciprocal via reciprocal_approx_fast (~5x faster than reciprocal)
  - denominator broadcast: one fp32r matmul per head pair (block-diag
    ones [2,128] x rq [2,qn] -> [128,qn])

Device dataflow per (qchunk, head-pair):
  for kc: S^T[k,q] x2 heads (PE, concurrent row tiles)
          E = exp(S^T/8) both heads in one ACT op (bf16 out)
          O'[65,q] += [V|1].T @ E  per head (PE, PSUM accum)
  rq = recip(O'[64]) x2 (DVE approx)
  bcast = blockdiag_ones.T @ rq  (PE, fp32r)
  on[128,q] = [O'_A[0:64]; O'_B[0:64]] * bcast   (DVE x2)
then grouped o_proj per q-tile (PE bf16, row-packed ic pairs) + bias (DVE).
"""

import numpy as np
import ml_dtypes

import concourse.bass as bass
import concourse.mybir as mybir
import concourse.tile as tile
from concourse import bacc
from concourse.bass_utils import run_bass_kernel_spmd

f32 = mybir.dt.float32
f32r = mybir.dt.float32r
bf16 = mybir.dt.bfloat16
BF = ml_dtypes.bfloat16

B, SQ, SK, D, H, HD, G, GD = 4, 1024, 2048, 1024, 16, 64, 4, 256
NCORE = 8
DS = D // 2          # dims per core (8 heads)
HPC = 8              # heads per core
P = 128

TRACE = False        # test.py sets kernel.TRACE = True for profiling
LAST_RUN = {}        # test.py reads exec_time_ns etc. from here

_CACHE = {}


def _pad_up(n, m):
    return ((n + m - 1) // m) * m


def _chunks(total, step):
    out = []
    q0 = 0
    while q0 < total:
        take = min(step, total - q0)
        out.append((q0, take))
        q0 += take
    return out


def build_nc(sqp, skp):
    """Build the per-core Bass program for padded shapes [sqp, skp]."""
    nkc = skp // P
    qchunks = _chunks(sqp, 512)

    nc = bacc.Bacc("TRN2", target_bir_lowering=False, debug=False,
                   num_devices=NCORE)

    qt_d = nc.dram_tensor("qt", [DS, sqp], bf16, kind="ExternalInput")
    kt_d = nc.dram_tensor("kt", [DS, skp], bf16, kind="ExternalInput")
    va_d = nc.dram_tensor("va", [skp, HPC * (HD + 1)], bf16,
                          kind="ExternalInput")
    wt_d = nc.dram_tensor("wt", [2, 2, P, GD], bf16, kind="ExternalInput")
    bb_d = nc.dram_tensor("bb", [P, DS], f32, kind="ExternalInput")
    out_d = nc.dram_tensor("out", [sqp, DS], f32, kind="ExternalOutput")

    with tile.TileContext(nc) as tc:
        with (
            tc.tile_pool(name="big", bufs=1) as big,
            tc.tile_pool(name="consts", bufs=1) as consts,
            tc.tile_pool(name="e_pool", bufs=3) as e_pool,
            tc.tile_pool(name="on_pool", bufs=2) as on_pool,
            tc.tile_pool(name="rq_pool", bufs=2) as rq_pool,
            tc.tile_pool(name="fo_pool", bufs=3) as fo_pool,
            tc.tile_pool(name="ps_s_pool", bufs=2, space="PSUM") as ps_s_pool,
            tc.tile_pool(name="ps_o_pool", bufs=2, space="PSUM") as ps_o_pool,
            tc.tile_pool(name="ps_x_pool", bufs=2, space="PSUM") as ps_x_pool,
        ):
            # ---- static loads ----
            kt_s, qt_s = [], []
            for j in range(4):
                t = big.tile([P, skp], bf16, tag=f"kt{j}")
                nc.sync.dma_start(out=t, in_=kt_d[j * P:(j + 1) * P, :])
                kt_s.append(t)
                t = big.tile([P, sqp], bf16, tag=f"qt{j}")
                nc.sync.dma_start(out=t, in_=qt_d[j * P:(j + 1) * P, :])
                qt_s.append(t)
            va_r = va_d.rearrange("(kc p) x -> kc p x", p=P)
            va_s = []
            for kc in range(nkc):
                t = big.tile([P, HPC, HD + 1], bf16, tag=f"va{kc}")
                nc.sync.dma_start(
                    out=t,
                    in_=va_r[kc].rearrange("p (h d) -> p h d", h=HPC))
                va_s.append(t)
            wt_s = []
            for g in range(2):
                row = []
                for i in range(2):
                    t = consts.tile([P, GD], bf16, tag=f"wt{g}{i}")
                    nc.sync.dma_start(out=t, in_=wt_d[g, i])
                    row.append(t)
                wt_s.append(row)
            bb_s = consts.tile([P, DS], f32)
            nc.sync.dma_start(out=bb_s, in_=bb_d[:, :])
            # all-ones [33, 64]: rows 0 and 32 are the lhsT for the two
            # recip-broadcast outer products (bases must match the rhs)
            ones33 = consts.tile([33, HD], bf16)
            nc.vector.memset(ones33, 1.0)


            # ---- main loops ----
            for q0, qn in qchunks:
                on_tiles = []
                for pr in range(4):
                    hA, hB = 2 * pr, 2 * pr + 1
                    ps_oA = ps_o_pool.tile([HD + 1, qn], f32, tag="ps_o")
                    ps_oB = ps_o_pool.tile([HD + 1, qn], f32, tag="ps_o")
                    for kc in range(nkc):
                        ps_s = ps_s_pool.tile([P, 2, 512], f32, tag="ps_s")
                        nc.tensor.matmul(
                            ps_s[:, 0, :qn],
                            kt_s[pr][0:HD, kc * P:(kc + 1) * P],
                            qt_s[pr][0:HD, q0:q0 + qn],
                            start=True, stop=True)
                        nc.tensor.matmul(
                            ps_s[:, 1, :qn],
                            kt_s[pr][HD:2 * HD, kc * P:(kc + 1) * P],
                            qt_s[pr][HD:2 * HD, q0:q0 + qn],
                            start=True, stop=True)
                        e = e_pool.tile([P, 2, qn], bf16, tag="e")
                        nc.scalar.activation(
                            e[:, :, :], ps_s[:, :, :qn],
                            mybir.ActivationFunctionType.Exp,
                            scale=0.125)
                        nc.tensor.matmul(
                            ps_oA[:, :], va_s[kc][:, hA, :], e[:, 0, :],
                            start=(kc == 0), stop=(kc == nkc - 1))
                        nc.tensor.matmul(
                            ps_oB[:, :], va_s[kc][:, hB, :], e[:, 1, :],
                            start=(kc == 0), stop=(kc == nkc - 1))
                    rq = rq_pool.tile([33, qn], f32, tag="rq")
                    nc.vector.reciprocal_approx_fast(
                        rq[0:1, :], ps_oA[HD:HD + 1, :])
                    nc.vector.reciprocal_approx_fast(
                        rq[32:33, :], ps_oB[HD:HD + 1, :])
                    rqr = rq_pool.tile([33, qn], bf16, tag="rqr")
                    nc.vector.tensor_copy(rqr[:, :], rq[:, :])
                    ps_b = ps_x_pool.tile([P, qn], f32, tag="ps_x",
                                          padded_shape=[P, 512])
                    nc.tensor.matmul(
                        ps_b[0:HD, :], ones33[0:1, :],
                        rqr[0:1, :], start=True, stop=True)
                    nc.tensor.matmul(
                        ps_b[HD:2 * HD, :], ones33[32:33, :],
                        rqr[32:33, :], start=True, stop=True)
                    bc = rq_pool.tile([P, qn], bf16, tag="bc")
                    nc.vector.tensor_copy(bc[:, :], ps_b[:, :])
                    on = on_pool.tile([P, qn], bf16, tag=f"on{pr}",
                                      padded_shape=[P, 512])
                    nc.vector.tensor_mul(
                        on[0:HD, :], ps_oA[0:HD, :], bc[0:HD, :])
                    nc.vector.tensor_mul(
                        on[HD:2 * HD, :], ps_oB[0:HD, :], bc[HD:2 * HD, :])
                    on_tiles.append(on)

                for t0, tn in _chunks(qn, P):
                    fo = fo_pool.tile([P, DS], f32, tag="fo")
                    for g in range(2):
                        ps_out = ps_x_pool.tile([P, GD], f32, tag="ps_x",
                                                padded_shape=[P, 512])
                        for i, pr in enumerate((2 * g, 2 * g + 1)):
                            for half in range(2):
                                nc.tensor.matmul(
                                    ps_out[0:tn, :],
                                    on_tiles[pr][HD * half:HD * (half + 1),
                                                 t0:t0 + tn],
                                    wt_s[g][i][HD * half:HD * (half + 1), :],
                                    start=(i == 0 and half == 0),
                                    stop=(i == 1 and half == 1))
                        nc.vector.tensor_add(
                            fo[0:tn, g * GD:(g + 1) * GD], ps_out[0:tn, :],
                            bb_s[0:tn, g * GD:(g + 1) * GD])
                    nc.sync.dma_start(
                        out=out_d[q0 + t0: q0 + t0 + tn, :],
                        in_=fo[0:tn, :])
    nc.compile()
    return nc


def _prep_core_inputs(c, sqp, skp, q_idx, k_idx, query, key, value,
                      o_weight, o_bias):
    """Build the per-core input map. q_idx/k_idx are the compressed row
    indices per batch."""
    b, s = c // 2, c % 2
    dsl = slice(s * DS, (s + 1) * DS)

    qi = q_idx[b]
    ki = k_idx[b]
    nq, nk = len(qi), len(ki)

    qt = np.zeros((DS, sqp), BF)
    qt[:, :nq] = query[b][qi][:, dsl].T.astype(BF)
    kt = np.zeros((DS, skp), BF)
    kt[:, :nk] = key[b][ki][:, dsl].T.astype(BF)
    va = np.zeros((skp, HPC, HD + 1), BF)
    va[:nk, :, :HD] = value[b][ki][:, dsl].reshape(nk, HPC, HD).astype(BF)
    va[:nk, :, HD] = 1.0
    va = va.reshape(skp, HPC * (HD + 1))

    # o_proj weights for groups 2s, 2s+1; per group two [128, 256] tiles
    # holding W^T input-chunk pairs (ic0|ic1) and (ic2|ic3).
    wt = np.zeros((2, 2, P, GD), BF)
    for g in range(2):
        wtg = o_weight[2 * s + g].T.reshape(4, HD, GD)   # [ic, 64, 256]
        wt[g, 0] = np.concatenate([wtg[0], wtg[1]], axis=0).astype(BF)
        wt[g, 1] = np.concatenate([wtg[2], wtg[3]], axis=0).astype(BF)
    bb = np.broadcast_to(o_bias[dsl].astype(np.float32), (P, DS))
    return {"qt": np.ascontiguousarray(qt), "kt": np.ascontiguousarray(kt),
            "va": np.ascontiguousarray(va),
            "wt": np.ascontiguousarray(wt),
            "bb": np.ascontiguousarray(bb)}


def kernel(query, key, value, key_mask, query_mask, o_weight, o_bias):
    query = np.asarray(query, np.float32)
    key = np.asarray(key, np.float32)
    value = np.asarray(value, np.float32)
    key_mask = np.asarray(key_mask)
    query_mask = np.asarray(query_mask)
    o_weight = np.asarray(o_weight, np.float32)
    o_bias = np.asarray(o_bias, np.float32)

    k_idx = [np.nonzero(key_mask[b, :, 0])[0] for b in range(B)]
    q_idx = [np.nonzero(query_mask[b, :, 0])[0] for b in range(B)]
    skp = max(P, _pad_up(max(len(i) for i in k_idx), P))
    sqp = max(64, _pad_up(max(len(i) for i in q_idx), 64))

    if (sqp, skp) not in _CACHE:
        _CACHE[(sqp, skp)] = build_nc(sqp, skp)
    nc = _CACHE[(sqp, skp)]

    in_maps = [
        _prep_core_inputs(c, sqp, skp, q_idx, k_idx, query, key, value,
                          o_weight, o_bias)
        for c in range(NCORE)
    ]
    res = run_bass_kernel_spmd(nc, in_maps, core_ids=list(range(NCORE)),
                               trace=TRACE)
    LAST_RUN["exec_time_ns"] = res.exec_time_ns
    LAST_RUN["profile_json"] = res.profile_json
    LAST_RUN["results"] = res

    out = np.empty((B, SQ, D), np.float32)
    for c in range(NCORE):
        b, s = c // 2, c % 2
        core_out = res.results[c]["out"]              # [sqp, DS]
        qi = q_idx[b]
        out[b, :, s * DS:(s + 1) * DS] = o_bias[s * DS:(s + 1) * DS]
        out[b, qi, s * DS:(s + 1) * DS] = core_out[:len(qi)]
    return out
